# revision 19
# baseline (speedup 1.0000x reference)
"""AttGRU cell on 8 TRN2 NeuronCores.

Math (per reference):
    agg = einsum('ij,bj->bi', adj, x)                  # [B, N]
    r   = sigmoid(agg + h @ W_hr.T + b_hr)
    z   = sigmoid(agg + h @ W_hz.T + b_hz)
    n   = tanh(agg + r * (h @ W_hn.T + b_hn))
    out = (1 - z) * n + z * h

B=8, N=4096. Memory-bound: the four [N, N] f32 matrices (256 MB) dominate.

Sharding: row-shard adj/W_* over 8 cores (512 output features per core),
replicate x/h (tiny). Each core computes its 512 output columns; the host
concatenates. No collectives.

Design (v5):
- Gate-major weight streaming (adj -> W_hr -> W_hn -> W_hz): each gate's
  epilogue overlaps the next gate's DMA stream; only the z tail is serial.
- One [128, 5632] bf16 slab (11 contraction chunks) per DMA, all on the
  sync HWDGE ring; uniform cadence keeps the stream at HBM rate and the
  PE trailing by at most one slab.
- agg is folded into the z PSUM accumulator by an identity matmul, so the
  tail is sigmoid(psum) -> z*d -> +n -> out DMA.
- The final z slab is fetched as 3 sub-DMAs (4+4+3 chunks) so the PE
  trails the last transfer by only ~3 chunks.
- tanh(u) = 2*sigmoid(2u)-1 keeps ScalarE on a single activation table.
- bf16 weights halve HBM traffic vs f32 and stream at 1 cycle/row on the
  PE (f32 is 4 cycles/row); accumulation stays f32 in PSUM. rel err ~1.3e-3.

Per-core inputs (host-prepared):
  wall [12, 128, 5632] bf16 - per gate (adj, Whr, Whn, Whz): the sharded,
       transposed matrix as 33 contraction chunks of [128, 512] (chunk 32
       is the bias row-chunk so biases ride the matmul), 11 chunks/slab.
  vt   [128, 528] bf16 - stationary operand: [x.T | h.T] per chunk
       ([128, 16]); chunk 32 is [0 | ones-row] to activate the biases.
  hloc [8, 512] f32 - h column shard for the output blend.
  eye  [8, 8] f32 - identity, for folding agg into the z accumulator.
"""

from contextlib import ExitStack

import ml_dtypes
import numpy as np

import concourse.bass as bass
import concourse.tile as tile
from concourse import bacc, mybir
from concourse.bass_utils import run_bass_kernel_spmd

B = 8
N = 4096
NCORES = 8
S = N // NCORES          # 512 output cols per core
KC = 128                 # contraction chunk (PE partition dim)
NK = N // KC             # 32 data chunks
NKB = NK + 1             # +1 bias chunk
NCHUNKS = 4 * NKB        # 132 chunks across the 4 gates
CHUNKS_PER_SLAB = 6      # 132 = 22 * 6
NSLABS = NCHUNKS // CHUNKS_PER_SLAB      # 22
SLABW = CHUNKS_PER_SLAB * S              # 3072
M2 = 2 * B               # 16: [x | h] stationary columns
FINAL_SPLITS = (3, 3)    # final slab sub-DMA chunk counts
ZH = S // 2              # tail chain computed in column halves

BF16 = mybir.dt.bfloat16
F32 = mybir.dt.float32

_CACHED_NC = None


def _build():
    nc = bacc.Bacc(
        "TRN2",
        target_bir_lowering=False,
        debug=False,
        num_devices=NCORES,
    )
    wall = nc.dram_tensor("wall", [NSLABS, KC, SLABW], BF16, kind="ExternalInput")
    vt = nc.dram_tensor("vt", [KC, NKB * M2], BF16, kind="ExternalInput")
    hloc = nc.dram_tensor("hloc", [B, S], F32, kind="ExternalInput")
    eye = nc.dram_tensor("eye", [B, B], F32, kind="ExternalInput")
    out = nc.dram_tensor("out", [B, S], F32, kind="ExternalOutput")

    AF = mybir.ActivationFunctionType
    ALU = mybir.AluOpType

    with tile.TileContext(nc) as tc, ExitStack() as ctx:
        wpool = ctx.enter_context(tc.tile_pool(name="wall", bufs=3))
        cpool = ctx.enter_context(tc.tile_pool(name="const", bufs=1))
        ppool = ctx.enter_context(tc.tile_pool(name="acc", bufs=1, space="PSUM"))
        epool = ctx.enter_context(tc.tile_pool(name="epi", bufs=1))

        # vt on the sync ring (fast completion; the first matmul needs it),
        # the rest on gpsimd SWDGE (needed much later)
        vt_sb = cpool.tile([KC, NKB * M2], BF16, tag="vt")
        nc.sync.dma_start(vt_sb[:], vt[:])
        hloc_sb = cpool.tile([B, S], F32, tag="hloc")
        nc.gpsimd.dma_start(hloc_sb[:], hloc[:])
        eye_sb = cpool.tile([B, B], F32, tag="eye")
        nc.gpsimd.dma_start(eye_sb[:], eye[:])

        acc = [
            ppool.tile([B, S], F32, tag=f"acc{g}", name=f"acc{g}") for g in range(4)
        ]

        # epilogue tiles, declared up front
        s_agg = epool.tile([B, S], F32, tag="sagg")
        t_r = epool.tile([B, S], F32, tag="tr")
        r_t = epool.tile([B, S], F32, tag="r")
        t_n = epool.tile([B, S], F32, tag="tn")
        t_n2 = epool.tile([B, S], F32, tag="tn2")
        sg_t = epool.tile([B, S], F32, tag="sg")
        n_t = epool.tile([B, S], F32, tag="n")
        d_t = epool.tile([B, S], F32, tag="d")
        z_t = epool.tile([B, S], F32, tag="z")
        zd_t = epool.tile([B, S], F32, tag="zd")
        o_t = epool.tile([B, S], F32, tag="o")

        def vt_x(k):
            return vt_sb[:, k * M2 : k * M2 + B]

        def vt_h(k):
            return vt_sb[:, k * M2 + B : (k + 1) * M2]

        # one continuous stream of 132 chunks (gate-major order:
        # adj, W_hr, W_hn, W_hz), 6 chunks per slab; gate boundaries fall
        # mid-slab, which is fine - matmuls just switch accumulators
        for sl in range(NSLABS):
            wt = wpool.tile([KC, SLABW], BF16, tag="wt", name=f"wt{sl}")
            if sl == NSLABS - 1:
                # final slab: sub-DMAs so the PE trails by ~3 chunks
                c0 = 0
                for nsplit in FINAL_SPLITS:
                    nc.sync.dma_start(
                        wt[:, c0 * S : (c0 + nsplit) * S],
                        wall[sl][:, c0 * S : (c0 + nsplit) * S],
                    )
                    c0 += nsplit
            else:
                nc.sync.dma_start(wt[:], wall[sl])
            for c in range(CHUNKS_PER_SLAB):
                gc = sl * CHUNKS_PER_SLAB + c
                g, k = divmod(gc, NKB)
                if g == 3 and k == 0:
                    # open the z accumulation group by folding agg in
                    nc.tensor.matmul(
                        acc[3][:, :], eye_sb[:, :], s_agg[:, :],
                        start=True, stop=False,
                    )
                nc.tensor.matmul(
                    acc[g][:, :],
                    vt_x(k) if g == 0 else vt_h(k),
                    wt[:, c * S : (c + 1) * S],
                    start=(k == 0 and g != 3),
                    stop=(k == NKB - 1),
                )
                if k != NKB - 1:
                    continue
                # end of gate g: emit its epilogue; Tile starts each op as
                # soon as its deps clear, overlapping the ongoing stream
                if g == 0:
                    nc.vector.tensor_copy(s_agg[:], acc[0][:, :])
                elif g == 1:
                    nc.vector.tensor_add(t_r[:], acc[1][:, :], s_agg[:])
                    nc.scalar.activation(r_t[:], t_r[:], AF.Sigmoid)
                elif g == 2:
                    nc.vector.tensor_mul(t_n[:], acc[2][:, :], r_t[:])
                    nc.vector.tensor_add(t_n2[:], t_n[:], s_agg[:])
                    # tanh(u) = 2*sigmoid(2u) - 1 (keeps ACT on one table)
                    nc.scalar.activation(sg_t[:], t_n2[:], AF.Sigmoid, scale=2.0)
                    nc.vector.tensor_scalar(
                        n_t[:], sg_t[:], 2.0, 1.0, ALU.mult, ALU.subtract
                    )
                    nc.vector.tensor_sub(d_t[:], hloc_sb[:], n_t[:])
                else:
                    # z tail in column halves: pipelines ACT/DVE and the
                    # two out-DMA completions
                    for hf in range(2):
                        cols = slice(hf * ZH, (hf + 1) * ZH)
                        nc.scalar.activation(
                            z_t[:, cols], acc[3][:, cols], AF.Sigmoid
                        )
                        nc.vector.tensor_mul(
                            zd_t[:, cols], z_t[:, cols], d_t[:, cols]
                        )
                        nc.vector.tensor_add(
                            o_t[:, cols], zd_t[:, cols], n_t[:, cols]
                        )
                        nc.sync.dma_start(out[:, cols], o_t[:, cols])

    nc.compile()
    return nc


def _get_nc():
    global _CACHED_NC
    if _CACHED_NC is None:
        _CACHED_NC = _build()
    return _CACHED_NC


def make_in_maps(x, h, adj, W_hr, b_hr, W_hz, b_hz, W_hn, b_hn):
    bf = ml_dtypes.bfloat16
    x = np.asarray(x, np.float32)
    h = np.asarray(h, np.float32)
    adj = np.asarray(adj, np.float32)
    W_hr = np.asarray(W_hr, np.float32)
    W_hz = np.asarray(W_hz, np.float32)
    W_hn = np.asarray(W_hn, np.float32)
    b_hr = np.asarray(b_hr, np.float32)
    b_hz = np.asarray(b_hz, np.float32)
    b_hn = np.asarray(b_hn, np.float32)

    vt_full = np.zeros((NKB * KC, M2), np.float32)
    vt_full[:N, :B] = x.T
    vt_full[:N, B:] = h.T
    vt_full[N, B:] = 1.0  # bias-chunk ones row (h side only)
    vt_packed = np.ascontiguousarray(
        vt_full.reshape(NKB, KC, M2).transpose(1, 0, 2).reshape(KC, NKB * M2)
    ).astype(bf)

    in_maps = []
    for s in range(NCORES):
        rs, re = s * S, (s + 1) * S
        # stream order: adj, W_hr, W_hn, W_hz (z last -> shortest tail)
        chunks = []
        for W, b in (
            (adj, None),
            (W_hr, b_hr),
            (W_hn, b_hn),
            (W_hz, b_hz),
        ):
            gm = np.zeros((NKB * KC, S), np.float32)
            gm[:N] = W[rs:re].T
            if b is not None:
                gm[N] = b[rs:re]
            chunks.append(gm.reshape(NKB, KC, S))
        wallp = np.ascontiguousarray(
            np.concatenate(chunks, axis=0)            # [132, 128, 512]
            .reshape(NSLABS, CHUNKS_PER_SLAB, KC, S)
            .transpose(0, 2, 1, 3)
            .reshape(NSLABS, KC, SLABW)
        ).astype(bf)
        in_maps.append(
            {
                "wall": wallp,
                "vt": vt_packed,
                "hloc": np.ascontiguousarray(h[:, rs:re]),
                "eye": np.eye(B, dtype=np.float32),
            }
        )
    return in_maps


def run(in_maps, trace=False, **kw):
    nc = _get_nc()
    return run_bass_kernel_spmd(
        nc, in_maps, core_ids=list(range(NCORES)), trace=trace, **kw
    )


def kernel(x, h, adj, W_hr, b_hr, W_hz, b_hz, W_hn, b_hn):
    in_maps = make_in_maps(x, h, adj, W_hr, b_hr, W_hz, b_hz, W_hn, b_hn)
    res = run(in_maps)
    return np.concatenate(
        [np.asarray(res.results[s]["out"]) for s in range(NCORES)], axis=1
    )


# revision 20
# speedup vs baseline: 1.0583x; 1.0583x over previous
"""AttGRU cell on 8 TRN2 NeuronCores.

Math (per reference):
    agg = einsum('ij,bj->bi', adj, x)                  # [B, N]
    r   = sigmoid(agg + h @ W_hr.T + b_hr)
    z   = sigmoid(agg + h @ W_hz.T + b_hz)
    n   = tanh(agg + r * (h @ W_hn.T + b_hn))
    out = (1 - z) * n + z * h

B=8, N=4096. Memory-bound: the four [N, N] f32 matrices (256 MB) dominate.

Sharding: row-shard adj/W_* over 8 cores (512 output features per core),
replicate x/h (tiny). Each core computes its 512 output columns; the host
concatenates. No collectives.

Design (v5):
- Gate-major weight streaming (adj -> W_hr -> W_hn -> W_hz): each gate's
  epilogue overlaps the next gate's DMA stream; only the z tail is serial.
- One [128, 5632] bf16 slab (11 contraction chunks) per DMA, all on the
  sync HWDGE ring; uniform cadence keeps the stream at HBM rate and the
  PE trailing by at most one slab.
- agg is folded into the z PSUM accumulator by an identity matmul, so the
  tail is sigmoid(psum) -> z*d -> +n -> out DMA.
- The final z slab is fetched as 3 sub-DMAs (4+4+3 chunks) so the PE
  trails the last transfer by only ~3 chunks.
- tanh(u) = 2*sigmoid(2u)-1 keeps ScalarE on a single activation table.
- bf16 weights halve HBM traffic vs f32 and stream at 1 cycle/row on the
  PE (f32 is 4 cycles/row); accumulation stays f32 in PSUM. rel err ~1.3e-3.

Per-core inputs (host-prepared):
  wall [12, 128, 5632] bf16 - per gate (adj, Whr, Whn, Whz): the sharded,
       transposed matrix as 33 contraction chunks of [128, 512] (chunk 32
       is the bias row-chunk so biases ride the matmul), 11 chunks/slab.
  vt   [128, 528] bf16 - stationary operand: [x.T | h.T] per chunk
       ([128, 16]); chunk 32 is [0 | ones-row] to activate the biases.
  hloc [8, 512] f32 - h column shard for the output blend.
  eye  [8, 8] f32 - identity, for folding agg into the z accumulator.
"""

from contextlib import ExitStack

import ml_dtypes
import numpy as np

import concourse.bass as bass
import concourse.tile as tile
from concourse import bacc, mybir
from concourse.bass_utils import run_bass_kernel_spmd

B = 8
N = 4096
NCORES = 8
S = N // NCORES          # 512 output cols per core
KC = 128                 # contraction chunk (PE partition dim)
NK = N // KC             # 32 data chunks
NKB = NK + 1             # +1 bias chunk
NCHUNKS = 4 * NKB        # 132 chunks across the 4 gates
CHUNKS_PER_SLAB = 6      # 132 = 22 * 6
NSLABS = NCHUNKS // CHUNKS_PER_SLAB      # 22
SLABW = CHUNKS_PER_SLAB * S              # 3072
M2 = 2 * B               # 16: [x | h] stationary columns
FINAL_SPLITS = (3, 3)    # final slab sub-DMA chunk counts
ZH = S // 2              # tail chain computed in column halves

BF16 = mybir.dt.bfloat16
F32 = mybir.dt.float32

_CACHED_NC = None


def _build():
    nc = bacc.Bacc(
        "TRN2",
        target_bir_lowering=False,
        debug=False,
        num_devices=NCORES,
    )
    wall = nc.dram_tensor("wall", [NSLABS, KC, SLABW], BF16, kind="ExternalInput")
    vt = nc.dram_tensor("vt", [KC, NKB * M2], BF16, kind="ExternalInput")
    hloc = nc.dram_tensor("hloc", [B, S], F32, kind="ExternalInput")
    eye = nc.dram_tensor("eye", [B, B], F32, kind="ExternalInput")
    out = nc.dram_tensor("out", [B, S], F32, kind="ExternalOutput")

    AF = mybir.ActivationFunctionType
    ALU = mybir.AluOpType

    with tile.TileContext(nc) as tc, ExitStack() as ctx:
        wpool = ctx.enter_context(tc.tile_pool(name="wall", bufs=8))
        cpool = ctx.enter_context(tc.tile_pool(name="const", bufs=1))
        ppool = ctx.enter_context(tc.tile_pool(name="acc", bufs=1, space="PSUM"))
        epool = ctx.enter_context(tc.tile_pool(name="epi", bufs=1))

        # vt on the sync ring (fast completion; the first matmul needs it),
        # the rest on gpsimd SWDGE (needed much later)
        vt_sb = cpool.tile([KC, NKB * M2], BF16, tag="vt")
        nc.sync.dma_start(vt_sb[:], vt[:])
        hloc_sb = cpool.tile([B, S], F32, tag="hloc")
        nc.gpsimd.dma_start(hloc_sb[:], hloc[:])
        eye_sb = cpool.tile([B, B], F32, tag="eye")
        nc.gpsimd.dma_start(eye_sb[:], eye[:])

        acc = [
            ppool.tile([B, S], F32, tag=f"acc{g}", name=f"acc{g}") for g in range(4)
        ]

        # epilogue tiles, declared up front
        s_agg = epool.tile([B, S], F32, tag="sagg")
        t_r = epool.tile([B, S], F32, tag="tr")
        r_t = epool.tile([B, S], F32, tag="r")
        t_n = epool.tile([B, S], F32, tag="tn")
        t_n2 = epool.tile([B, S], F32, tag="tn2")
        sg_t = epool.tile([B, S], F32, tag="sg")
        n_t = epool.tile([B, S], F32, tag="n")
        d_t = epool.tile([B, S], F32, tag="d")
        z_t = epool.tile([B, S], F32, tag="z")
        zd_t = epool.tile([B, S], F32, tag="zd")
        o_t = epool.tile([B, S], F32, tag="o")

        def vt_x(k):
            return vt_sb[:, k * M2 : k * M2 + B]

        def vt_h(k):
            return vt_sb[:, k * M2 + B : (k + 1) * M2]

        # one continuous stream of 132 chunks (gate-major order:
        # adj, W_hr, W_hn, W_hz), 6 chunks per slab; gate boundaries fall
        # mid-slab, which is fine - matmuls just switch accumulators
        for sl in range(NSLABS):
            wt = wpool.tile([KC, SLABW], BF16, tag="wt", name=f"wt{sl}")
            if sl == NSLABS - 1:
                # final slab: sub-DMAs so the PE trails by ~3 chunks
                c0 = 0
                for nsplit in FINAL_SPLITS:
                    nc.sync.dma_start(
                        wt[:, c0 * S : (c0 + nsplit) * S],
                        wall[sl][:, c0 * S : (c0 + nsplit) * S],
                    )
                    c0 += nsplit
            else:
                nc.sync.dma_start(wt[:], wall[sl])
            for c in range(CHUNKS_PER_SLAB):
                gc = sl * CHUNKS_PER_SLAB + c
                g, k = divmod(gc, NKB)
                if g == 3 and k == 0:
                    # open the z accumulation group by folding agg in
                    nc.tensor.matmul(
                        acc[3][:, :], eye_sb[:, :], s_agg[:, :],
                        start=True, stop=False,
                    )
                nc.tensor.matmul(
                    acc[g][:, :],
                    vt_x(k) if g == 0 else vt_h(k),
                    wt[:, c * S : (c + 1) * S],
                    start=(k == 0 and g != 3),
                    stop=(k == NKB - 1),
                )
                if k != NKB - 1:
                    continue
                # end of gate g: emit its epilogue; Tile starts each op as
                # soon as its deps clear, overlapping the ongoing stream
                if g == 0:
                    nc.vector.tensor_copy(s_agg[:], acc[0][:, :])
                elif g == 1:
                    nc.vector.tensor_add(t_r[:], acc[1][:, :], s_agg[:])
                    nc.scalar.activation(r_t[:], t_r[:], AF.Sigmoid)
                elif g == 2:
                    nc.vector.tensor_mul(t_n[:], acc[2][:, :], r_t[:])
                    nc.vector.tensor_add(t_n2[:], t_n[:], s_agg[:])
                    # tanh(u) = 2*sigmoid(2u) - 1 (keeps ACT on one table)
                    nc.scalar.activation(sg_t[:], t_n2[:], AF.Sigmoid, scale=2.0)
                    nc.vector.tensor_scalar(
                        n_t[:], sg_t[:], 2.0, 1.0, ALU.mult, ALU.subtract
                    )
                    nc.vector.tensor_sub(d_t[:], hloc_sb[:], n_t[:])
                else:
                    # z tail in column halves: pipelines ACT/DVE and the
                    # two out-DMA completions
                    for hf in range(2):
                        cols = slice(hf * ZH, (hf + 1) * ZH)
                        nc.scalar.activation(
                            z_t[:, cols], acc[3][:, cols], AF.Sigmoid
                        )
                        nc.vector.tensor_mul(
                            zd_t[:, cols], z_t[:, cols], d_t[:, cols]
                        )
                        nc.vector.tensor_add(
                            o_t[:, cols], zd_t[:, cols], n_t[:, cols]
                        )
                        nc.sync.dma_start(out[:, cols], o_t[:, cols])

    nc.compile()
    return nc


def _get_nc():
    global _CACHED_NC
    if _CACHED_NC is None:
        _CACHED_NC = _build()
    return _CACHED_NC


def make_in_maps(x, h, adj, W_hr, b_hr, W_hz, b_hz, W_hn, b_hn):
    bf = ml_dtypes.bfloat16
    x = np.asarray(x, np.float32)
    h = np.asarray(h, np.float32)
    adj = np.asarray(adj, np.float32)
    W_hr = np.asarray(W_hr, np.float32)
    W_hz = np.asarray(W_hz, np.float32)
    W_hn = np.asarray(W_hn, np.float32)
    b_hr = np.asarray(b_hr, np.float32)
    b_hz = np.asarray(b_hz, np.float32)
    b_hn = np.asarray(b_hn, np.float32)

    vt_full = np.zeros((NKB * KC, M2), np.float32)
    vt_full[:N, :B] = x.T
    vt_full[:N, B:] = h.T
    vt_full[N, B:] = 1.0  # bias-chunk ones row (h side only)
    vt_packed = np.ascontiguousarray(
        vt_full.reshape(NKB, KC, M2).transpose(1, 0, 2).reshape(KC, NKB * M2)
    ).astype(bf)

    in_maps = []
    for s in range(NCORES):
        rs, re = s * S, (s + 1) * S
        # stream order: adj, W_hr, W_hn, W_hz (z last -> shortest tail)
        chunks = []
        for W, b in (
            (adj, None),
            (W_hr, b_hr),
            (W_hn, b_hn),
            (W_hz, b_hz),
        ):
            gm = np.zeros((NKB * KC, S), np.float32)
            gm[:N] = W[rs:re].T
            if b is not None:
                gm[N] = b[rs:re]
            chunks.append(gm.reshape(NKB, KC, S))
        wallp = np.ascontiguousarray(
            np.concatenate(chunks, axis=0)            # [132, 128, 512]
            .reshape(NSLABS, CHUNKS_PER_SLAB, KC, S)
            .transpose(0, 2, 1, 3)
            .reshape(NSLABS, KC, SLABW)
        ).astype(bf)
        in_maps.append(
            {
                "wall": wallp,
                "vt": vt_packed,
                "hloc": np.ascontiguousarray(h[:, rs:re]),
                "eye": np.eye(B, dtype=np.float32),
            }
        )
    return in_maps


def run(in_maps, trace=False, **kw):
    nc = _get_nc()
    return run_bass_kernel_spmd(
        nc, in_maps, core_ids=list(range(NCORES)), trace=trace, **kw
    )


def kernel(x, h, adj, W_hr, b_hr, W_hz, b_hz, W_hn, b_hn):
    in_maps = make_in_maps(x, h, adj, W_hr, b_hr, W_hz, b_hz, W_hn, b_hn)
    res = run(in_maps)
    return np.concatenate(
        [np.asarray(res.results[s]["out"]) for s in range(NCORES)], axis=1
    )


# revision 23
# speedup vs baseline: 1.1424x; 1.0794x over previous
"""AttGRU cell on 8 TRN2 NeuronCores.

Math (per reference):
    agg = einsum('ij,bj->bi', adj, x)                  # [B, N]
    r   = sigmoid(agg + h @ W_hr.T + b_hr)
    z   = sigmoid(agg + h @ W_hz.T + b_hz)
    n   = tanh(agg + r * (h @ W_hn.T + b_hn))
    out = (1 - z) * n + z * h

B=8, N=4096. Memory-bound: the four [N, N] f32 matrices (256 MB) dominate.

Sharding: row-shard adj/W_* over 8 cores (512 output features per core),
replicate x/h (tiny). Each core computes its 512 output columns; the host
concatenates. No collectives.

Design (v5):
- Gate-major weight streaming (adj -> W_hr -> W_hn -> W_hz): each gate's
  epilogue overlaps the next gate's DMA stream; only the z tail is serial.
- One [128, 5632] bf16 slab (11 contraction chunks) per DMA, all on the
  sync HWDGE ring; uniform cadence keeps the stream at HBM rate and the
  PE trailing by at most one slab.
- agg is folded into the z PSUM accumulator by an identity matmul, so the
  tail is sigmoid(psum) -> z*d -> +n -> out DMA.
- The final z slab is fetched as 3 sub-DMAs (4+4+3 chunks) so the PE
  trails the last transfer by only ~3 chunks.
- tanh(u) = 2*sigmoid(2u)-1 keeps ScalarE on a single activation table.
- bf16 weights halve HBM traffic vs f32 and stream at 1 cycle/row on the
  PE (f32 is 4 cycles/row); accumulation stays f32 in PSUM. rel err ~1.3e-3.

Per-core inputs (host-prepared):
  wall [12, 128, 5632] bf16 - per gate (adj, Whr, Whn, Whz): the sharded,
       transposed matrix as 33 contraction chunks of [128, 512] (chunk 32
       is the bias row-chunk so biases ride the matmul), 11 chunks/slab.
  vt   [128, 528] bf16 - stationary operand: [x.T | h.T] per chunk
       ([128, 16]); chunk 32 is [0 | ones-row] to activate the biases.
  hloc [8, 512] f32 - h column shard for the output blend.
  eye  [8, 8] f32 - identity, for folding agg into the z accumulator.
"""

from contextlib import ExitStack

import ml_dtypes
import numpy as np

import concourse.bass as bass
import concourse.tile as tile
from concourse import bacc, mybir
from concourse.bass_utils import run_bass_kernel_spmd

B = 8
N = 4096
NCORES = 8
S = N // NCORES          # 512 output cols per core
KC = 128                 # contraction chunk (PE partition dim)
NK = N // KC             # 32 data chunks
NKB = NK + 1             # +1 bias chunk
NCHUNKS = 4 * NKB        # 132 chunks across the 4 gates
CHUNKS_PER_SLAB = 11     # 132 = 12 * 11
NSLABS = NCHUNKS // CHUNKS_PER_SLAB      # 12
SLABW = CHUNKS_PER_SLAB * S              # 5632
M2 = 2 * B               # 16: [x | h] stationary columns
FINAL_SPLITS = (4, 4, 3)  # sub-DMA chunk counts for the last two slabs
N_SPLIT_SLABS = 2        # how many trailing slabs get sub-DMAs
ZH = S // 2              # tail chain computed in column halves

BF16 = mybir.dt.bfloat16
F32 = mybir.dt.float32

_CACHED_NC = None


def _build():
    nc = bacc.Bacc(
        "TRN2",
        target_bir_lowering=False,
        debug=False,
        num_devices=NCORES,
    )
    wall = nc.dram_tensor("wall", [NSLABS, KC, SLABW], BF16, kind="ExternalInput")
    vt = nc.dram_tensor("vt", [KC, NKB * M2], BF16, kind="ExternalInput")
    hloc = nc.dram_tensor("hloc", [B, S], F32, kind="ExternalInput")
    eye = nc.dram_tensor("eye", [B, B], F32, kind="ExternalInput")
    out = nc.dram_tensor("out", [B, S], F32, kind="ExternalOutput")

    AF = mybir.ActivationFunctionType
    ALU = mybir.AluOpType

    with tile.TileContext(nc) as tc, ExitStack() as ctx:
        wpool = ctx.enter_context(tc.tile_pool(name="wall", bufs=3))
        cpool = ctx.enter_context(tc.tile_pool(name="const", bufs=1))
        ppool = ctx.enter_context(tc.tile_pool(name="acc", bufs=1, space="PSUM"))
        epool = ctx.enter_context(tc.tile_pool(name="epi", bufs=1))

        # vt on the sync ring (fast completion; the first matmul needs it),
        # the rest on gpsimd SWDGE (needed much later)
        vt_sb = cpool.tile([KC, NKB * M2], BF16, tag="vt")
        nc.sync.dma_start(vt_sb[:], vt[:])
        hloc_sb = cpool.tile([B, S], F32, tag="hloc")
        nc.gpsimd.dma_start(hloc_sb[:], hloc[:])
        eye_sb = cpool.tile([B, B], F32, tag="eye")
        nc.gpsimd.dma_start(eye_sb[:], eye[:])

        acc = [
            ppool.tile([B, S], F32, tag=f"acc{g}", name=f"acc{g}") for g in range(4)
        ]

        # epilogue tiles, declared up front
        s_agg = epool.tile([B, S], F32, tag="sagg")
        t_r = epool.tile([B, S], F32, tag="tr")
        r_t = epool.tile([B, S], F32, tag="r")
        t_n = epool.tile([B, S], F32, tag="tn")
        t_n2 = epool.tile([B, S], F32, tag="tn2")
        sg_t = epool.tile([B, S], F32, tag="sg")
        n_t = epool.tile([B, S], F32, tag="n")
        d_t = epool.tile([B, S], F32, tag="d")
        z_t = epool.tile([B, S], F32, tag="z")
        zd_t = epool.tile([B, S], F32, tag="zd")
        o_t = epool.tile([B, S], F32, tag="o")

        def vt_x(k):
            return vt_sb[:, k * M2 : k * M2 + B]

        def vt_h(k):
            return vt_sb[:, k * M2 + B : (k + 1) * M2]

        # one continuous stream of 132 chunks (gate-major order:
        # adj, W_hr, W_hn, W_hz), 6 chunks per slab; gate boundaries fall
        # mid-slab, which is fine - matmuls just switch accumulators
        for sl in range(NSLABS):
            wt = wpool.tile([KC, SLABW], BF16, tag="wt", name=f"wt{sl}")
            if sl >= NSLABS - N_SPLIT_SLABS:
                # trailing slabs: sub-DMAs so the PE trails by ~3 chunks
                c0 = 0
                for nsplit in FINAL_SPLITS:
                    nc.sync.dma_start(
                        wt[:, c0 * S : (c0 + nsplit) * S],
                        wall[sl][:, c0 * S : (c0 + nsplit) * S],
                    )
                    c0 += nsplit
            else:
                nc.sync.dma_start(wt[:], wall[sl])
            for c in range(CHUNKS_PER_SLAB):
                gc = sl * CHUNKS_PER_SLAB + c
                g, k = divmod(gc, NKB)
                if g == 3 and k == 0:
                    # open the z accumulation group by folding agg in
                    nc.tensor.matmul(
                        acc[3][:, :], eye_sb[:, :], s_agg[:, :],
                        start=True, stop=False,
                    )
                nc.tensor.matmul(
                    acc[g][:, :],
                    vt_x(k) if g == 0 else vt_h(k),
                    wt[:, c * S : (c + 1) * S],
                    start=(k == 0 and g != 3),
                    stop=(k == NKB - 1),
                )
                if k != NKB - 1:
                    continue
                # end of gate g: emit its epilogue; Tile starts each op as
                # soon as its deps clear, overlapping the ongoing stream
                if g == 0:
                    nc.vector.tensor_copy(s_agg[:], acc[0][:, :])
                elif g == 1:
                    nc.vector.tensor_add(t_r[:], acc[1][:, :], s_agg[:])
                    nc.scalar.activation(r_t[:], t_r[:], AF.Sigmoid)
                elif g == 2:
                    nc.vector.tensor_mul(t_n[:], acc[2][:, :], r_t[:])
                    nc.vector.tensor_add(t_n2[:], t_n[:], s_agg[:])
                    # tanh(u) = 2*sigmoid(2u) - 1 (keeps ACT on one table)
                    nc.scalar.activation(sg_t[:], t_n2[:], AF.Sigmoid, scale=2.0)
                    nc.vector.tensor_scalar(
                        n_t[:], sg_t[:], 2.0, 1.0, ALU.mult, ALU.subtract
                    )
                    nc.vector.tensor_sub(d_t[:], hloc_sb[:], n_t[:])
                else:
                    # z tail in column halves: pipelines ACT/DVE and the
                    # two out-DMA completions
                    for hf in range(2):
                        cols = slice(hf * ZH, (hf + 1) * ZH)
                        nc.scalar.activation(
                            z_t[:, cols], acc[3][:, cols], AF.Sigmoid
                        )
                        nc.vector.tensor_mul(
                            zd_t[:, cols], z_t[:, cols], d_t[:, cols]
                        )
                        nc.vector.tensor_add(
                            o_t[:, cols], zd_t[:, cols], n_t[:, cols]
                        )
                        nc.sync.dma_start(out[:, cols], o_t[:, cols])

    nc.compile()
    return nc


def _get_nc():
    global _CACHED_NC
    if _CACHED_NC is None:
        _CACHED_NC = _build()
    return _CACHED_NC


def make_in_maps(x, h, adj, W_hr, b_hr, W_hz, b_hz, W_hn, b_hn):
    bf = ml_dtypes.bfloat16
    x = np.asarray(x, np.float32)
    h = np.asarray(h, np.float32)
    adj = np.asarray(adj, np.float32)
    W_hr = np.asarray(W_hr, np.float32)
    W_hz = np.asarray(W_hz, np.float32)
    W_hn = np.asarray(W_hn, np.float32)
    b_hr = np.asarray(b_hr, np.float32)
    b_hz = np.asarray(b_hz, np.float32)
    b_hn = np.asarray(b_hn, np.float32)

    vt_full = np.zeros((NKB * KC, M2), np.float32)
    vt_full[:N, :B] = x.T
    vt_full[:N, B:] = h.T
    vt_full[N, B:] = 1.0  # bias-chunk ones row (h side only)
    vt_packed = np.ascontiguousarray(
        vt_full.reshape(NKB, KC, M2).transpose(1, 0, 2).reshape(KC, NKB * M2)
    ).astype(bf)

    in_maps = []
    for s in range(NCORES):
        rs, re = s * S, (s + 1) * S
        # stream order: adj, W_hr, W_hn, W_hz (z last -> shortest tail)
        chunks = []
        for W, b in (
            (adj, None),
            (W_hr, b_hr),
            (W_hn, b_hn),
            (W_hz, b_hz),
        ):
            gm = np.zeros((NKB * KC, S), np.float32)
            gm[:N] = W[rs:re].T
            if b is not None:
                gm[N] = b[rs:re]
            chunks.append(gm.reshape(NKB, KC, S))
        wallp = np.ascontiguousarray(
            np.concatenate(chunks, axis=0)            # [132, 128, 512]
            .reshape(NSLABS, CHUNKS_PER_SLAB, KC, S)
            .transpose(0, 2, 1, 3)
            .reshape(NSLABS, KC, SLABW)
        ).astype(bf)
        in_maps.append(
            {
                "wall": wallp,
                "vt": vt_packed,
                "hloc": np.ascontiguousarray(h[:, rs:re]),
                "eye": np.eye(B, dtype=np.float32),
            }
        )
    return in_maps


def run(in_maps, trace=False, **kw):
    nc = _get_nc()
    return run_bass_kernel_spmd(
        nc, in_maps, core_ids=list(range(NCORES)), trace=trace, **kw
    )


def kernel(x, h, adj, W_hr, b_hr, W_hz, b_hz, W_hn, b_hn):
    in_maps = make_in_maps(x, h, adj, W_hr, b_hr, W_hz, b_hz, W_hn, b_hn)
    res = run(in_maps)
    return np.concatenate(
        [np.asarray(res.results[s]["out"]) for s in range(NCORES)], axis=1
    )


# revision 28
# speedup vs baseline: 1.2735x; 1.1147x over previous
"""AttGRU cell on 8 TRN2 NeuronCores.

Math (per reference):
    agg = einsum('ij,bj->bi', adj, x)                  # [B, N]
    r   = sigmoid(agg + h @ W_hr.T + b_hr)
    z   = sigmoid(agg + h @ W_hz.T + b_hz)
    n   = tanh(agg + r * (h @ W_hn.T + b_hn))
    out = (1 - z) * n + z * h

B=8, N=4096. Memory-bound: the four [N, N] f32 matrices (256 MB) dominate.

Sharding: row-shard adj/W_* over 8 cores (512 output features per core),
replicate x/h (tiny). Each core computes its 512 output columns; the host
concatenates. No collectives.

Design (v5):
- Gate-major weight streaming (adj -> W_hr -> W_hn -> W_hz): each gate's
  epilogue overlaps the next gate's DMA stream; only the z tail is serial.
- One [128, 5632] bf16 slab (11 contraction chunks) per DMA, all on the
  sync HWDGE ring; uniform cadence keeps the stream at HBM rate and the
  PE trailing by at most one slab.
- agg is folded into the z PSUM accumulator by an identity matmul, so the
  tail is sigmoid(psum) -> z*d -> +n -> out DMA.
- The final z slab is fetched as 3 sub-DMAs (4+4+3 chunks) so the PE
  trails the last transfer by only ~3 chunks.
- tanh(u) = 2*sigmoid(2u)-1 keeps ScalarE on a single activation table.
- bf16 weights halve HBM traffic vs f32 and stream at 1 cycle/row on the
  PE (f32 is 4 cycles/row); accumulation stays f32 in PSUM. rel err ~1.3e-3.

Per-core inputs (host-prepared):
  wall [12, 128, 5632] bf16 - per gate (adj, Whr, Whn, Whz): the sharded,
       transposed matrix as 33 contraction chunks of [128, 512] (chunk 32
       is the bias row-chunk so biases ride the matmul), 11 chunks/slab.
  vt   [128, 528] bf16 - stationary operand: [x.T | h.T] per chunk
       ([128, 16]); chunk 32 is [0 | ones-row] to activate the biases.
  hloc [8, 512] f32 - h column shard for the output blend.
  eye  [8, 8] f32 - identity, for folding agg into the z accumulator.
"""

from contextlib import ExitStack

import ml_dtypes
import numpy as np

import concourse.bass as bass
import concourse.tile as tile
from concourse import bacc, mybir
from concourse.bass_utils import run_bass_kernel_spmd

B = 8
N = 4096
NCORES = 8
S = N // NCORES          # 512 output cols per core
KC = 128                 # contraction chunk (PE partition dim)
NK = N // KC             # 32 data chunks
NKB = NK + 1             # +1 bias chunk
CHUNKS_PER_SLAB = 11     # 33 chunks per gate = 3 slabs of 11
NSLABS_BF = 9            # bf16 slabs: gates W_hr, W_hn, W_hz
NSLABS_A = 3             # fp8 slabs: adj
SLABW = CHUNKS_PER_SLAB * S              # 5632
M2 = 2 * B               # 16 (host vt packing only)
FINAL_SPLITS = (4, 4, 3)  # sub-DMA chunk counts for the last two slabs
N_SPLIT_SLABS = 2        # how many trailing bf16 slabs get sub-DMAs
ZH = S // 2              # tail chain computed in column halves
ADJ_SCALE = 4096.0       # adj pre-scale so fp8-e4m3 doesn't flush to zero

BF16 = mybir.dt.bfloat16
F32 = mybir.dt.float32
FP8 = mybir.dt.float8e4

_CACHED_NC = None


def _build():
    nc = bacc.Bacc(
        "TRN2",
        target_bir_lowering=False,
        debug=False,
        num_devices=NCORES,
    )
    adjw = nc.dram_tensor("adjw", [NSLABS_A, KC, SLABW], FP8, kind="ExternalInput")
    wall = nc.dram_tensor("wall", [NSLABS_BF, KC, SLABW], BF16, kind="ExternalInput")
    vtx = nc.dram_tensor("vtx", [KC, NKB * B], FP8, kind="ExternalInput")
    vth = nc.dram_tensor("vth", [KC, NKB * B], BF16, kind="ExternalInput")
    hloc = nc.dram_tensor("hloc", [B, S], F32, kind="ExternalInput")
    eye = nc.dram_tensor("eye", [B, B], F32, kind="ExternalInput")
    out = nc.dram_tensor("out", [B, S], F32, kind="ExternalOutput")

    AF = mybir.ActivationFunctionType
    ALU = mybir.AluOpType

    with tile.TileContext(nc) as tc, ExitStack() as ctx:
        wpool = ctx.enter_context(tc.tile_pool(name="wall", bufs=3))
        cpool = ctx.enter_context(tc.tile_pool(name="const", bufs=1))
        ppool = ctx.enter_context(tc.tile_pool(name="acc", bufs=1, space="PSUM"))
        epool = ctx.enter_context(tc.tile_pool(name="epi", bufs=1))

        # vtx on the sync ring (fast completion; the first matmul needs it),
        # the rest on gpsimd SWDGE (needed later)
        vtx_sb = cpool.tile([KC, NKB * B], FP8, tag="vtx")
        nc.sync.dma_start(vtx_sb[:], vtx[:])
        vth_sb = cpool.tile([KC, NKB * B], BF16, tag="vth")
        nc.gpsimd.dma_start(vth_sb[:], vth[:])
        hloc_sb = cpool.tile([B, S], F32, tag="hloc")
        nc.gpsimd.dma_start(hloc_sb[:], hloc[:])
        eye_sb = cpool.tile([B, B], F32, tag="eye")
        nc.gpsimd.dma_start(eye_sb[:], eye[:])

        acc = [
            ppool.tile([B, S], F32, tag=f"acc{g}", name=f"acc{g}") for g in range(4)
        ]

        # epilogue tiles, declared up front
        s_agg = epool.tile([B, S], F32, tag="sagg")
        t_r = epool.tile([B, S], F32, tag="tr")
        r_t = epool.tile([B, S], F32, tag="r")
        t_n = epool.tile([B, S], F32, tag="tn")
        t_n2 = epool.tile([B, S], F32, tag="tn2")
        sg_t = epool.tile([B, S], F32, tag="sg")
        n_t = epool.tile([B, S], F32, tag="n")
        d_t = epool.tile([B, S], F32, tag="d")
        z_t = epool.tile([B, S], F32, tag="z")
        zd_t = epool.tile([B, S], F32, tag="zd")
        o_t = epool.tile([B, S], F32, tag="o")

        def vt_x(k):
            return vtx_sb[:, k * B : (k + 1) * B]

        def vt_h(k):
            return vth_sb[:, k * B : (k + 1) * B]

        # adj stream: 3 fp8 slabs (gate 0)
        for sl in range(NSLABS_A):
            wa = wpool.tile([KC, SLABW], FP8, tag="wa", name=f"wa{sl}")
            nc.sync.dma_start(wa[:], adjw[sl])
            for c in range(CHUNKS_PER_SLAB):
                k = sl * CHUNKS_PER_SLAB + c
                nc.tensor.matmul(
                    acc[0][:, :],
                    vt_x(k),
                    wa[:, c * S : (c + 1) * S],
                    start=(k == 0),
                    stop=(k == NKB - 1),
                )
        # descale agg (adj was pre-scaled by ADJ_SCALE for fp8 range)
        nc.vector.tensor_scalar_mul(s_agg[:], acc[0][:, :], 1.0 / ADJ_SCALE)

        # bf16 stream: gates 1=W_hr, 2=W_hn, 3=W_hz, 11 chunks per slab
        for sl in range(NSLABS_BF):
            wt = wpool.tile([KC, SLABW], BF16, tag="wt", name=f"wt{sl}")
            if sl >= NSLABS_BF - N_SPLIT_SLABS:
                # trailing slabs: sub-DMAs so the PE trails by ~3 chunks
                c0 = 0
                for nsplit in FINAL_SPLITS:
                    nc.sync.dma_start(
                        wt[:, c0 * S : (c0 + nsplit) * S],
                        wall[sl][:, c0 * S : (c0 + nsplit) * S],
                    )
                    c0 += nsplit
            else:
                nc.sync.dma_start(wt[:], wall[sl])
            for c in range(CHUNKS_PER_SLAB):
                gc = sl * CHUNKS_PER_SLAB + c
                g, k = divmod(gc, NKB)
                g += 1
                if g == 3 and k == 0:
                    # open the z accumulation group by folding agg in
                    nc.tensor.matmul(
                        acc[3][:, :], eye_sb[:, :], s_agg[:, :],
                        start=True, stop=False,
                    )
                nc.tensor.matmul(
                    acc[g][:, :],
                    vt_h(k),
                    wt[:, c * S : (c + 1) * S],
                    start=(k == 0 and g != 3),
                    stop=(k == NKB - 1),
                )
                if k != NKB - 1:
                    continue
                # end of gate g: emit its epilogue; Tile starts each op as
                # soon as its deps clear, overlapping the ongoing stream
                if g == 1:
                    nc.vector.tensor_add(t_r[:], acc[1][:, :], s_agg[:])
                    nc.scalar.activation(r_t[:], t_r[:], AF.Sigmoid)
                elif g == 2:
                    nc.vector.tensor_mul(t_n[:], acc[2][:, :], r_t[:])
                    nc.vector.tensor_add(t_n2[:], t_n[:], s_agg[:])
                    # tanh(u) = 2*sigmoid(2u) - 1 (keeps ACT on one table)
                    nc.scalar.activation(sg_t[:], t_n2[:], AF.Sigmoid, scale=2.0)
                    nc.vector.tensor_scalar(
                        n_t[:], sg_t[:], 2.0, 1.0, ALU.mult, ALU.subtract
                    )
                    nc.vector.tensor_sub(d_t[:], hloc_sb[:], n_t[:])
                else:
                    # z tail in column halves: pipelines ACT/DVE and the
                    # two out-DMA completions
                    for hf in range(2):
                        cols = slice(hf * ZH, (hf + 1) * ZH)
                        nc.scalar.activation(
                            z_t[:, cols], acc[3][:, cols], AF.Sigmoid
                        )
                        nc.vector.tensor_mul(
                            zd_t[:, cols], z_t[:, cols], d_t[:, cols]
                        )
                        nc.vector.tensor_add(
                            o_t[:, cols], zd_t[:, cols], n_t[:, cols]
                        )
                        nc.sync.dma_start(out[:, cols], o_t[:, cols])

    nc.compile()
    return nc


def _get_nc():
    global _CACHED_NC
    if _CACHED_NC is None:
        _CACHED_NC = _build()
    return _CACHED_NC


def make_in_maps(x, h, adj, W_hr, b_hr, W_hz, b_hz, W_hn, b_hn):
    bf = ml_dtypes.bfloat16
    x = np.asarray(x, np.float32)
    h = np.asarray(h, np.float32)
    adj = np.asarray(adj, np.float32)
    W_hr = np.asarray(W_hr, np.float32)
    W_hz = np.asarray(W_hz, np.float32)
    W_hn = np.asarray(W_hn, np.float32)
    b_hr = np.asarray(b_hr, np.float32)
    b_hz = np.asarray(b_hz, np.float32)
    b_hn = np.asarray(b_hn, np.float32)

    fp8 = ml_dtypes.float8_e4m3fn

    def pack_vt(v, ones_row):
        full = np.zeros((NKB * KC, B), np.float32)
        full[:N] = v.T
        if ones_row:
            full[N] = 1.0  # bias-chunk ones row
        return np.ascontiguousarray(
            full.reshape(NKB, KC, B).transpose(1, 0, 2).reshape(KC, NKB * B)
        )

    vtx_packed = pack_vt(x, False).astype(fp8)
    vth_packed = pack_vt(h, True).astype(bf)

    def pack_slabs(chunks_2d, nslabs):
        return np.ascontiguousarray(
            chunks_2d.reshape(nslabs, CHUNKS_PER_SLAB, KC, S)
            .transpose(0, 2, 1, 3)
            .reshape(nslabs, KC, SLABW)
        )

    in_maps = []
    for s in range(NCORES):
        rs, re = s * S, (s + 1) * S
        gm = np.zeros((NKB * KC, S), np.float32)
        gm[:N] = adj[rs:re].T * ADJ_SCALE
        adjp = pack_slabs(gm, NSLABS_A).astype(fp8)

        # stream order: W_hr, W_hn, W_hz (z last -> shortest tail)
        chunks = []
        for W, b in ((W_hr, b_hr), (W_hn, b_hn), (W_hz, b_hz)):
            gm = np.zeros((NKB * KC, S), np.float32)
            gm[:N] = W[rs:re].T
            gm[N] = b[rs:re]
            chunks.append(gm)
        wallp = pack_slabs(np.concatenate(chunks, axis=0), NSLABS_BF).astype(bf)
        in_maps.append(
            {
                "adjw": adjp,
                "wall": wallp,
                "vtx": vtx_packed,
                "vth": vth_packed,
                "hloc": np.ascontiguousarray(h[:, rs:re]),
                "eye": np.eye(B, dtype=np.float32),
            }
        )
    return in_maps


def run(in_maps, trace=False, **kw):
    nc = _get_nc()
    return run_bass_kernel_spmd(
        nc, in_maps, core_ids=list(range(NCORES)), trace=trace, **kw
    )


def kernel(x, h, adj, W_hr, b_hr, W_hz, b_hz, W_hn, b_hn):
    in_maps = make_in_maps(x, h, adj, W_hr, b_hr, W_hz, b_hz, W_hn, b_hn)
    res = run(in_maps)
    return np.concatenate(
        [np.asarray(res.results[s]["out"]) for s in range(NCORES)], axis=1
    )
